# revision 18
# baseline (speedup 1.0000x reference)
"""Trainium2 Bass kernel for nn_LoraMoeBlock (MoE with per-expert LoRA adapters).

Strategy
--------
Math reformulation ("slot-mask" form): with top-k=2, write the combine as a sum
over two slots k in {0,1}; slot k of token t selects expert e_k(t) with weight
w_k(t).  Per-expert LoRA terms become dense matmuls over the packed [E*R=128]
axis by masking the small per-token LoRA activations:

    xa      = x @ A            (A packed [H, E*R]) -- all experts at once
    xam_k   = xa * M_k         (M_k[t, e*R+r] = 1 iff e == e_k(t))
    gate_k  = x @ Wg + xam_gk @ Bg      (PSUM accumulation, Bg pre-scaled by 2)
    up_k    = x @ Wu + xam_uk @ Bu
    wact_k  = w_k * silu(gate_k) * up_k
    actw    = wact_0 + wact_1
    z_k     = wact_k @ Ad ;  zm_k = z_k * M_k
    out     = actw @ Wd + zm_0 @ Bd + zm_1 @ Bd   (PSUM accumulation)

Everything is a dense matmul + a handful of elementwise passes over [T, Fc]
per slot; exact top-2 routing semantics are preserved via the masks, which are
computed on-device from an fp32 router (selection is precision sensitive).

Sharding: the F=4096 axis is split across the 8 cores (Fc=512 each).  Weights
touching F are sliced per core; x, router and LoRA-A weights are replicated.
Each core produces a partial out^T [H, T] (partial over its F slice, since the
down-projection contracts over F) plus the router logits; the host sums the 8
partials as the unshard step (the contraction-sharded equivalent of gather).

On-device layout is "L1": feature-on-partition, token-on-free ([*, T] tiles),
which lets every matmul consume weights in their natural layout with x^T (host
provides the transpose) as the shared moving operand.  The small router /
top-k stage runs in an L2 packed layout [128, (tile, e)] where softmax and
top-2 are free-dim reductions; masks hop back to L1 via one PE transpose and a
constant "expand" matmul per token-tile.

Matmul inputs are bf16 (full PE rate; fp32 matmul is 4x slower on trn2); the
router matmul is fp32 to make expert selection bit-reliable.  All PSUM
accumulation is fp32.
"""

import sys

sys.path.insert(0, "/opt/trn_rl_repo")

from contextlib import ExitStack

import ml_dtypes
import numpy as np

import concourse.bass as bass
import concourse.tile as tile
from concourse import bacc, mybir
from concourse.bass import ts
from concourse.bass_utils import run_bass_kernel_spmd

BF16 = ml_dtypes.bfloat16
F32 = mybir.dt.float32
BF = mybir.dt.bfloat16
AF = mybir.ActivationFunctionType
ALU = mybir.AluOpType

B, S, H, F, E, R = 1, 1024, 1024, 4096, 8, 16
T = B * S
ER = E * R  # 128
NCORES = 8
FC = F // NCORES  # 512 per-core F slice
SCALING = 2.0

P = 128
KT = H // P  # 8 contraction tiles over H
FT = FC // P  # 4 partition tiles over Fc
HT = H // P  # 8 output tiles over H
TT8 = T // P  # 8 token tiles
TN = 512  # moving-operand chunk
TC = T // TN  # 2 chunks


def _emit(nc, tc, ctx, d):
    """Emit the per-core program. d: dict of dram tensor handles."""
    PS = bass.MemorySpace.PSUM

    consts = ctx.enter_context(tc.tile_pool(name="consts", bufs=1))
    sb = ctx.enter_context(tc.tile_pool(name="sb", bufs=1))
    stage = ctx.enter_context(tc.tile_pool(name="stage", bufs=3))
    psg = ctx.enter_context(tc.tile_pool(name="psg", bufs=2, space=PS))
    psu = ctx.enter_context(tc.tile_pool(name="psu", bufs=2, space=PS))
    pso = ctx.enter_context(tc.tile_pool(name="pso", bufs=2, space=PS))
    psm = ctx.enter_context(tc.tile_pool(name="psm", bufs=2, space=PS))

    dma = nc.sync.dma_start

    # ---- constant loads -------------------------------------------------
    def cload(name, shape, dt, ap):
        t_ = consts.tile(shape, dt, tag=name, name=name)
        dma(t_[:], ap)
        return t_

    # DMA order = consumption order. The router is the head of the mask
    # critical path, so its inputs stream first (xT32 split into T-halves so
    # the first router chunk starts after ~2MB); xa/mains fill PE behind it.
    # x^T halves are separate tiles so chunk-0 compute only waits on the
    # chunk-0 DMA (one tile written by two DMAs would serialize on both)
    wrn = cload("wrn", [P, KT, 16], F32, d["wrn"].ap().rearrange("(k p) e -> p k e", p=P))
    xT32_src = d["xT32"].ap().rearrange("(k p) t -> p k t", p=P)
    xTbf_src = d["xTbf"].ap().rearrange("(k p) t -> p k t", p=P)
    xT32h, xTbfh = [None, None], [None, None]
    xT32h[0] = cload("xT32_0", [P, KT, TN], F32, xT32_src[:, :, ts(0, TN)])
    xTbfh[0] = cload("xTbf_0", [P, KT, TN], BF, xTbf_src[:, :, ts(0, TN)])
    xT32h[1] = cload("xT32_1", [P, KT, TN], F32, xT32_src[:, :, ts(1, TN)])
    xTbfh[1] = cload("xTbf_1", [P, KT, TN], BF, xTbf_src[:, :, ts(1, TN)])
    ag = cload("ag", [P, KT, ER], BF, d["ag"].ap().rearrange("(k p) r -> p k r", p=P))
    au = cload("au", [P, KT, ER], BF, d["au"].ap().rearrange("(k p) r -> p k r", p=P))
    noiseL2 = cload("noiseL2", [P, TT8, E], F32, d["noiseL2"].ap().rearrange("p (t e) -> p t e", e=E))
    emat4 = cload("emat4", [32, 4 * P], BF, d["emat4"].ap())
    i128 = cload("i128", [P, P], F32, d["i128"].ap())
    wg = cload("wg", [P, KT, FC], BF, d["wg"].ap().rearrange("(k p) f -> p k f", p=P))
    wu = cload("wu", [P, KT, FC], BF, d["wu"].ap().rearrange("(k p) f -> p k f", p=P))
    bg = cload("bg", [P, FC], BF, d["bg"].ap())
    bu = cload("bu", [P, FC], BF, d["bu"].ap())
    ad = cload("ad", [P, FT, ER], BF, d["ad"].ap().rearrange("(k p) r -> p k r", p=P))
    wd = cload("wd", [P, FT, H], BF, d["wd"].ap().rearrange("(k p) h -> p k h", p=P))
    bd = cload("bd", [P, H], BF, d["bd"].ap())

    # ---- B: router matmuls (fp32) + C1: xa (PE fill work) ----------------
    rn_sb = sb.tile([16, T], F32, tag="rn_sb")

    def router_chunk(c):
        ps = psm.tile([16, TN], F32, tag="ps_small", name="ps_rn")
        for k in range(KT):
            nc.tensor.matmul(ps[:], wrn[:, k, :], xT32h[c][:, k, :],
                             start=(k == 0), stop=(k == KT - 1))
        nc.scalar.copy(rn_sb[:, ts(c, TN)], ps[:])

    router_chunk(0)
    xa_sb = {}
    for nm, a_t in (("g", ag), ("u", au)):
        xa = sb.tile([P, T], BF, tag=f"xa_{nm}", name=f"xa_{nm}")
        for c in range(TC):
            ps = psg.tile([P, TN], F32, tag="ps_g", name="ps_xa")
            for k in range(KT):
                nc.tensor.matmul(ps[:], a_t[:, k, :], xTbfh[c][:, k, :],
                                 start=(k == 0), stop=(k == KT - 1))
            nc.scalar.copy(xa[:, ts(c, TN)], ps[:])
        xa_sb[nm] = xa
    router_chunk(1)

    # transpose to L2 packed layout [128, (tile, 16)]
    rn_L2 = sb.tile([P, TT8, 16], F32, tag="rn_L2")
    for tl in range(TT8):
        pst = psm.tile([P, 16], F32, tag="ps_small")
        nc.tensor.transpose(pst[:], rn_sb[:, ts(tl, P)], i128[:16, :16])
        nc.scalar.copy(rn_L2[:, tl, :], pst[:])

    # router_logits = logits + noise * softplus(noise_logits)
    # softplus(x) = ln(exp(x) + 1); |x| is small here so this is accurate and
    # Exp/Ln share one hardware activation table (Softplus has none).
    esp_t = sb.tile([P, TT8, E], F32, tag="esp_t")
    nc.scalar.activation(esp_t[:], rn_L2[:, :, 8:16], AF.Exp)
    sp_t = sb.tile([P, TT8, E], F32, tag="sp_t")
    nc.scalar.activation(sp_t[:], esp_t[:], AF.Ln, bias=1.0)
    nm_t = sb.tile([P, TT8, E], F32, tag="nm_t")
    nc.vector.tensor_tensor(nm_t[:], sp_t[:], noiseL2[:], ALU.mult)
    rl_L2 = sb.tile([P, TT8, E], F32, tag="rl_L2")
    nc.vector.tensor_tensor(rl_L2[:], nm_t[:], rn_L2[:, :, 0:8], ALU.add)
    dma(d["rl"].ap().rearrange("(t p) e -> p t e", p=P), rl_L2[:])

    # softmax top-2 (unnormalized exp works: softmax denominator cancels in
    # both the argmax and the renormalized top-2 weights)
    ex = sb.tile([P, TT8, E], F32, tag="ex")
    nc.scalar.activation(ex[:], rl_L2[:], AF.Exp)
    m1 = sb.tile([P, TT8], F32, tag="m1")
    nc.vector.tensor_reduce(m1[:], ex[:], axis=mybir.AxisListType.X, op=ALU.max)
    # MALL free layout: (tile, mask, e) with mask in {M0, M1, MW0, MW1}
    mall = sb.tile([P, TT8, 4, E], F32, tag="mall")
    m1b = m1[:, :, None].broadcast_to([P, TT8, E])
    nc.vector.tensor_tensor(mall[:, :, 0, :], ex[:], m1b, ALU.is_equal)
    pm = sb.tile([P, TT8, E], F32, tag="pm")
    nc.vector.scalar_tensor_tensor(pm[:], mall[:, :, 0, :], -1e30, ex[:],
                                   op0=ALU.mult, op1=ALU.add)
    m2 = sb.tile([P, TT8], F32, tag="m2")
    nc.vector.tensor_reduce(m2[:], pm[:], axis=mybir.AxisListType.X, op=ALU.max)
    m2b = m2[:, :, None].broadcast_to([P, TT8, E])
    nc.vector.tensor_tensor(mall[:, :, 1, :], pm[:], m2b, ALU.is_equal)
    s12 = sb.tile([P, TT8], F32, tag="s12")
    nc.vector.tensor_tensor(s12[:], m1[:], m2[:], ALU.add)
    r12 = sb.tile([P, TT8], F32, tag="r12")
    nc.vector.reciprocal(r12[:], s12[:])
    w1_ = sb.tile([P, TT8, 2], F32, tag="w1_")  # w per slot
    nc.vector.tensor_tensor(w1_[:, :, 0], m1[:], r12[:], ALU.mult)
    nc.vector.tensor_tensor(w1_[:, :, 1], m2[:], r12[:], ALU.mult)
    for k in range(2):
        wb = w1_[:, :, k][:, :, None].broadcast_to([P, TT8, E])
        nc.vector.tensor_tensor(mall[:, :, 2 + k, :], mall[:, :, k, :], wb, ALU.mult)

    # Runway: gate/up mains for two (f, c=0) pairs are emitted BEFORE the
    # mask transposes so the PE's in-order queue has mask-independent work
    # covering the DVE top-k latency (keeps HAM warm: no >3.4us PE gap).
    def emit_main(which, f, c):
        pool, wt = (psg, wg) if which == "g" else (psu, wu)
        ps = pool.tile([P, TN], F32, tag=f"ps_{which}", name=f"ps_{which}")
        for k in range(KT):
            nc.tensor.matmul(ps[:], wt[:, k, ts(f, P)], xTbfh[c][:, k, :],
                             start=(k == 0), stop=(k == KT - 1))
        mn = stage.tile([P, TN], BF, tag=f"main_{which}", name=f"main_{which}")
        nc.scalar.copy(mn[:], ps[:])  # main only (before the lora matmul)
        return ps, mn

    RUNWAY = [(0, 0), (1, 0)]
    pre = {}
    for f, c in RUNWAY:
        pre["g", f, c] = emit_main("g", f, c)
        pre["u", f, c] = emit_main("u", f, c)

    # transpose masks to L1 and expand over the (e, r) / broadcast axes.
    # M_exp_k[e*R+r, t] = M_k[t, e]; W_exp_k[f, t] = w_k[t].
    mexp = [sb.tile([P, T], BF, tag=f"mexp{k}", name=f"mexp{k}") for k in range(2)]
    wexp = [sb.tile([P, T], BF, tag=f"wexp{k}", name=f"wexp{k}") for k in range(2)]
    dest = [mexp[0], mexp[1], wexp[0], wexp[1]]
    for tl in range(TT8):
        pst = psm.tile([32, P], F32, tag="ps_small")
        nc.tensor.transpose(pst[:], mall[:, tl, :, :], i128[:, :])
        mt4 = stage.tile([32, P], BF, tag="mt4")
        nc.scalar.copy(mt4[:], pst[:])
        for m in range(4):
            pse = psm.tile([P, P], F32, tag="ps_small")
            nc.tensor.matmul(pse[:], emat4[:, ts(m, P)], mt4[:], start=True, stop=True)
            nc.scalar.copy(dest[m][:, ts(tl, P)], pse[:])

    xam = {}
    for nm in ("g", "u"):
        for k in range(2):
            xm = sb.tile([P, T], BF, tag=f"xam_{nm}{k}", name=f"xam_{nm}{k}")
            nc.vector.tensor_tensor(xm[:], xa_sb[nm][:], mexp[k][:], ALU.mult)
            xam[nm, k] = xm

    # ---- D/E/F/G chunk-major: the down matmuls for chunk 0 overlap the
    # gate/up/act chain for chunk 1 ---------------------------------------
    wact = [[sb.tile([P, T], BF, tag=f"wact{k}_{f}", name=f"wact{k}_{f}")
             for f in range(FT)] for k in range(2)]
    zm = [sb.tile([P, T], BF, tag=f"zm{k}", name=f"zm{k}") for k in range(2)]
    actw = [sb.tile([P, T], BF, tag=f"actw_{f}", name=f"actw_{f}") for f in range(FT)]

    def emit_lora0(which, f, c, ps):
        # slot-0 lora accumulates in PSUM on top of main (group reopened)
        b_t = bg if which == "g" else bu
        nc.tensor.matmul(ps[:], b_t[:, ts(f, P)], xam[which, 0][:, ts(c, TN)],
                         start=False, stop=True, skip_group_check=True)

    def emit_lora1(which, f, c, mn):
        b_t = bg if which == "g" else bu
        psl = psm.tile([P, TN], F32, tag="ps_small", name="ps_l")
        nc.tensor.matmul(psl[:], b_t[:, ts(f, P)], xam[which, 1][:, ts(c, TN)],
                         start=True, stop=True)
        t1 = stage.tile([P, TN], BF, tag=f"{which}b1", name=f"{which}b1")
        nc.vector.tensor_tensor(t1[:], psl[:], mn[:], ALU.add)
        return t1

    for c in range(TC):
        for f in range(FT):
            psG, mainG = pre.pop(("g", f, c)) if ("g", f, c) in pre else emit_main("g", f, c)
            emit_lora0("g", f, c, psG)
            psU, mainU = pre.pop(("u", f, c)) if ("u", f, c) in pre else emit_main("u", f, c)
            emit_lora0("u", f, c, psU)
            gb1 = emit_lora1("g", f, c, mainG)
            ub1 = emit_lora1("u", f, c, mainU)
            # slot 0: read gate/up straight from PSUM (saves two ACT evicts)
            sig0 = stage.tile([P, TN], BF, tag="sig")
            nc.scalar.activation(sig0[:], psG[:], AF.Sigmoid)
            gs0 = stage.tile([P, TN], BF, tag="gs")
            nc.vector.tensor_tensor(gs0[:], psG[:], sig0[:], ALU.mult)
            uw0 = stage.tile([P, TN], BF, tag="uw")
            nc.vector.tensor_tensor(uw0[:], psU[:], wexp[0][:, ts(c, TN)], ALU.mult)
            nc.vector.tensor_tensor(wact[0][f][:, ts(c, TN)], gs0[:], uw0[:], ALU.mult)
            # slot 1 — on GpSimd: the DVE saturates during this phase (it
            # gates PSUM-slot release and stalls the PE), GpSimd sits idle
            sig1 = stage.tile([P, TN], BF, tag="sig")
            nc.scalar.activation(sig1[:], gb1[:], AF.Sigmoid)
            gs1 = stage.tile([P, TN], BF, tag="gs")
            nc.gpsimd.tensor_tensor(gs1[:], gb1[:], sig1[:], ALU.mult)
            uw1 = stage.tile([P, TN], BF, tag="uw")
            nc.gpsimd.tensor_tensor(uw1[:], ub1[:], wexp[1][:, ts(c, TN)], ALU.mult)
            nc.gpsimd.tensor_tensor(wact[1][f][:, ts(c, TN)], gs1[:], uw1[:], ALU.mult)
        # z_k = wact_k @ Ad, masked (this chunk)
        for k in range(2):
            ps = psm.tile([P, TN], F32, tag="ps_small", name="ps_z")
            for f in range(FT):
                nc.tensor.matmul(ps[:], ad[:, f, :], wact[k][f][:, ts(c, TN)],
                                 start=(f == 0), stop=(f == FT - 1))
            nc.vector.tensor_tensor(zm[k][:, ts(c, TN)], ps[:], mexp[k][:, ts(c, TN)], ALU.mult)
        # fold slots
        for f in range(FT):
            nc.vector.tensor_tensor(actw[f][:, ts(c, TN)], wact[0][f][:, ts(c, TN)],
                                    wact[1][f][:, ts(c, TN)], ALU.add)
        # down projection + lora (this chunk)
        for h in range(HT):
            ps = pso.tile([P, TN], F32, tag="ps_o", name="ps_o")
            for f in range(FT):
                nc.tensor.matmul(ps[:], wd[:, f, ts(h, P)], actw[f][:, ts(c, TN)],
                                 start=(f == 0), stop=False)
            nc.tensor.matmul(ps[:], bd[:, ts(h, P)], zm[0][:, ts(c, TN)],
                             start=False, stop=False)
            nc.tensor.matmul(ps[:], bd[:, ts(h, P)], zm[1][:, ts(c, TN)],
                             start=False, stop=True)
            ot = stage.tile([P, TN], F32, tag="ot")
            nc.vector.tensor_copy(ot[:], ps[:])
            dma(d["outT"].ap()[ts(h, P), ts(c, TN)], ot[:])


def _build():
    nc = bacc.Bacc("TRN2", target_bir_lowering=False, debug=False)
    d = {}
    d["xT32"] = nc.dram_tensor("xT32", [H, T], F32, kind="ExternalInput")
    d["xTbf"] = nc.dram_tensor("xTbf", [H, T], BF, kind="ExternalInput")
    d["wrn"] = nc.dram_tensor("wrn", [H, 16], F32, kind="ExternalInput")
    d["wg"] = nc.dram_tensor("wg", [H, FC], BF, kind="ExternalInput")
    d["wu"] = nc.dram_tensor("wu", [H, FC], BF, kind="ExternalInput")
    d["wd"] = nc.dram_tensor("wd", [FC, H], BF, kind="ExternalInput")
    d["ag"] = nc.dram_tensor("ag", [H, ER], BF, kind="ExternalInput")
    d["au"] = nc.dram_tensor("au", [H, ER], BF, kind="ExternalInput")
    d["bg"] = nc.dram_tensor("bg", [ER, FC], BF, kind="ExternalInput")
    d["bu"] = nc.dram_tensor("bu", [ER, FC], BF, kind="ExternalInput")
    d["ad"] = nc.dram_tensor("ad", [FC, ER], BF, kind="ExternalInput")
    d["bd"] = nc.dram_tensor("bd", [ER, H], BF, kind="ExternalInput")
    d["noiseL2"] = nc.dram_tensor("noiseL2", [P, TT8 * E], F32, kind="ExternalInput")
    d["emat4"] = nc.dram_tensor("emat4", [32, 4 * P], BF, kind="ExternalInput")
    d["i128"] = nc.dram_tensor("i128", [P, P], F32, kind="ExternalInput")
    d["outT"] = nc.dram_tensor("outT", [H, T], F32, kind="ExternalOutput")
    d["rl"] = nc.dram_tensor("rl", [T, E], F32, kind="ExternalOutput")

    with tile.TileContext(nc) as tc, ExitStack() as ctx:
        _emit(nc, tc, ctx, d)
    nc.compile()
    return nc


_NC = None


def _get_nc():
    global _NC
    if _NC is None:
        _NC = _build()
    return _NC


def _emat4_np():
    m = np.zeros((32, 4 * P), dtype=np.float32)
    for blk in range(4):
        for e in range(E):
            if blk < 2:
                m[blk * 8 + e, blk * P + e * R:blk * P + (e + 1) * R] = 1.0
            else:
                m[blk * 8 + e, blk * P:(blk + 1) * P] = 1.0
    return m.astype(BF16)


def make_in_maps(hidden_states, noise, w_route, w_noise, w_gate, w_up, w_down,
                 a_gate, b_gate, a_up, b_up, a_down, b_down):
    x = np.asarray(hidden_states, dtype=np.float32).reshape(T, H)
    xT = np.ascontiguousarray(x.T)
    shared = {
        "xT32": xT,
        "xTbf": xT.astype(BF16),
        "wrn": np.ascontiguousarray(
            np.concatenate([w_route, w_noise], axis=1).astype(np.float32)),
        "ag": np.ascontiguousarray(
            a_gate.transpose(1, 0, 2).reshape(H, ER)).astype(BF16),
        "au": np.ascontiguousarray(
            a_up.transpose(1, 0, 2).reshape(H, ER)).astype(BF16),
        "bd": np.ascontiguousarray(
            (SCALING * b_down).reshape(ER, H)).astype(BF16),
        "noiseL2": np.ascontiguousarray(
            np.asarray(noise, np.float32).reshape(TT8, P, E)
            .transpose(1, 0, 2).reshape(P, TT8 * E)),
        "emat4": _emat4_np(),
        "i128": np.eye(P, dtype=np.float32),
    }
    bg2 = (SCALING * b_gate).reshape(ER, F)
    bu2 = (SCALING * b_up).reshape(ER, F)
    adf = a_down.transpose(1, 0, 2).reshape(F, ER)
    in_maps = []
    for c in range(NCORES):
        fsl = slice(c * FC, (c + 1) * FC)
        m = dict(shared)
        m["wg"] = np.ascontiguousarray(w_gate[:, fsl]).astype(BF16)
        m["wu"] = np.ascontiguousarray(w_up[:, fsl]).astype(BF16)
        m["wd"] = np.ascontiguousarray(w_down[fsl, :]).astype(BF16)
        m["bg"] = np.ascontiguousarray(bg2[:, fsl]).astype(BF16)
        m["bu"] = np.ascontiguousarray(bu2[:, fsl]).astype(BF16)
        m["ad"] = np.ascontiguousarray(adf[fsl, :]).astype(BF16)
        in_maps.append(m)
    return in_maps


def kernel(hidden_states, noise, w_route, w_noise, w_gate, w_up, w_down,
           a_gate, b_gate, a_up, b_up, a_down, b_down, _trace=False):
    in_maps = make_in_maps(hidden_states, noise, w_route, w_noise, w_gate,
                           w_up, w_down, a_gate, b_gate, a_up, b_up,
                           a_down, b_down)
    nc = _get_nc()
    res = run_bass_kernel_spmd(nc, in_maps, list(range(NCORES)), trace=_trace)
    outs = res.results
    outT = np.zeros((H, T), dtype=np.float32)
    for o in outs:
        outT += o["outT"]
    out = np.ascontiguousarray(outT.T).reshape(B, S, H)
    rl = np.ascontiguousarray(outs[0]["rl"]).astype(np.float32)
    if _trace:
        kernel._last_exec_time_ns = res.exec_time_ns
    return out, rl


# revision 21
# speedup vs baseline: 1.1265x; 1.1265x over previous
"""Trainium2 Bass kernel for nn_LoraMoeBlock (MoE with per-expert LoRA adapters).

Strategy
--------
Math reformulation ("slot-mask" form): with top-k=2, write the combine as a sum
over two slots k in {0,1}; slot k of token t selects expert e_k(t) with weight
w_k(t).  Per-expert LoRA terms become dense matmuls over the packed [E*R=128]
axis by masking the small per-token LoRA activations:

    xa      = x @ A            (A packed [H, E*R]) -- all experts at once
    xam_k   = xa * M_k         (M_k[t, e*R+r] = 1 iff e == e_k(t))
    gate_k  = x @ Wg + xam_gk @ Bg      (PSUM accumulation, Bg pre-scaled by 2)
    up_k    = x @ Wu + xam_uk @ Bu
    wact_k  = w_k * silu(gate_k) * up_k
    actw    = wact_0 + wact_1
    z_k     = wact_k @ Ad ;  zm_k = z_k * M_k
    out     = actw @ Wd + zm_0 @ Bd + zm_1 @ Bd   (PSUM accumulation)

Everything is a dense matmul + a handful of elementwise passes over [T, Fc]
per slot; exact top-2 routing semantics are preserved via the masks, which are
computed on-device from an fp32 router (selection is precision sensitive).

Sharding: the F=4096 axis is split across the 8 cores (Fc=512 each).  Weights
touching F are sliced per core; x, router and LoRA-A weights are replicated.
Each core produces a partial out^T [H, T] (partial over its F slice, since the
down-projection contracts over F) plus the router logits; the host sums the 8
partials as the unshard step (the contraction-sharded equivalent of gather).

On-device layout is "L1": feature-on-partition, token-on-free ([*, T] tiles),
which lets every matmul consume weights in their natural layout with x^T (host
provides the transpose) as the shared moving operand.  The small router /
top-k stage runs in an L2 packed layout [128, (tile, e)] where softmax and
top-2 are free-dim reductions; masks hop back to L1 via one PE transpose and a
constant "expand" matmul per token-tile.

Matmul inputs are bf16 (full PE rate; fp32 matmul is 4x slower on trn2); the
router matmul is fp32 to make expert selection bit-reliable.  All PSUM
accumulation is fp32.
"""

import sys

sys.path.insert(0, "/opt/trn_rl_repo")

from contextlib import ExitStack

import ml_dtypes
import numpy as np

import concourse.bass as bass
import concourse.tile as tile
from concourse import bacc, mybir
from concourse.bass import ts
from concourse.bass_utils import run_bass_kernel_spmd

BF16 = ml_dtypes.bfloat16
F32 = mybir.dt.float32
BF = mybir.dt.bfloat16
AF = mybir.ActivationFunctionType
ALU = mybir.AluOpType

USE_SILU = True  # HW has a Silu table; CoreSim doesn't implement it, so
                 # sim_check flips this to False (x*sigmoid(x), same math)


def _silu_fn():
    return AF.Silu if USE_SILU else AF.Sigmoid


B, S, H, F, E, R = 1, 1024, 1024, 4096, 8, 16
T = B * S
ER = E * R  # 128
NCORES = 8
FC = F // NCORES  # 512 per-core F slice
SCALING = 2.0

P = 128
KT = H // P  # 8 contraction tiles over H
FT = FC // P  # 4 partition tiles over Fc
HT = H // P  # 8 output tiles over H
TT8 = T // P  # 8 token tiles
TN = 512  # moving-operand chunk
TC = T // TN  # 2 chunks


def _emit(nc, tc, ctx, d):
    """Emit the per-core program. d: dict of dram tensor handles."""
    PS = bass.MemorySpace.PSUM

    consts = ctx.enter_context(tc.tile_pool(name="consts", bufs=1))
    sb = ctx.enter_context(tc.tile_pool(name="sb", bufs=1))
    stage = ctx.enter_context(tc.tile_pool(name="stage", bufs=3))
    psg = ctx.enter_context(tc.tile_pool(name="psg", bufs=2, space=PS))
    psu = ctx.enter_context(tc.tile_pool(name="psu", bufs=2, space=PS))
    pso = ctx.enter_context(tc.tile_pool(name="pso", bufs=2, space=PS))
    psm = ctx.enter_context(tc.tile_pool(name="psm", bufs=2, space=PS))

    dma = nc.sync.dma_start

    # ---- constant loads -------------------------------------------------
    def cload(name, shape, dt, ap):
        t_ = consts.tile(shape, dt, tag=name, name=name)
        dma(t_[:], ap)
        return t_

    # DMA order = consumption order. The router is the head of the mask
    # critical path, so its inputs stream first (xT32 split into T-halves so
    # the first router chunk starts after ~2MB); xa/mains fill PE behind it.
    # x^T halves are separate tiles so chunk-0 compute only waits on the
    # chunk-0 DMA (one tile written by two DMAs would serialize on both)
    wrn = cload("wrn", [P, KT, 16], F32, d["wrn"].ap().rearrange("(k p) e -> p k e", p=P))
    xT32_src = d["xT32"].ap().rearrange("(k p) t -> p k t", p=P)
    xTbf_src = d["xTbf"].ap().rearrange("(k p) t -> p k t", p=P)
    xT32h, xTbfh = [None, None], [None, None]
    xT32h[0] = cload("xT32_0", [P, KT, TN], F32, xT32_src[:, :, ts(0, TN)])
    xTbfh[0] = cload("xTbf_0", [P, KT, TN], BF, xTbf_src[:, :, ts(0, TN)])
    xT32h[1] = cload("xT32_1", [P, KT, TN], F32, xT32_src[:, :, ts(1, TN)])
    xTbfh[1] = cload("xTbf_1", [P, KT, TN], BF, xTbf_src[:, :, ts(1, TN)])
    ag = cload("ag", [P, KT, ER], BF, d["ag"].ap().rearrange("(k p) r -> p k r", p=P))
    au = cload("au", [P, KT, ER], BF, d["au"].ap().rearrange("(k p) r -> p k r", p=P))
    noiseL2 = cload("noiseL2", [P, TT8, E], F32, d["noiseL2"].ap().rearrange("p (t e) -> p t e", e=E))
    emat4 = cload("emat4", [32, 4 * P], BF, d["emat4"].ap())
    i128 = cload("i128", [P, P], F32, d["i128"].ap())
    wg = cload("wg", [P, KT, FC], BF, d["wg"].ap().rearrange("(k p) f -> p k f", p=P))
    wu = cload("wu", [P, KT, FC], BF, d["wu"].ap().rearrange("(k p) f -> p k f", p=P))
    bg = cload("bg", [P, FC], BF, d["bg"].ap())
    bu = cload("bu", [P, FC], BF, d["bu"].ap())
    ad = cload("ad", [P, FT, ER], BF, d["ad"].ap().rearrange("(k p) r -> p k r", p=P))
    wd = cload("wd", [P, FT, H], BF, d["wd"].ap().rearrange("(k p) h -> p k h", p=P))
    bd = cload("bd", [P, H], BF, d["bd"].ap())

    # ---- B: router matmuls (fp32) + C1: xa (PE fill work) ----------------
    rn_sb = sb.tile([16, T], F32, tag="rn_sb")

    def router_chunk(c):
        ps = psm.tile([16, TN], F32, tag="ps_small", name="ps_rn")
        for k in range(KT):
            nc.tensor.matmul(ps[:], wrn[:, k, :], xT32h[c][:, k, :],
                             start=(k == 0), stop=(k == KT - 1))
        nc.scalar.copy(rn_sb[:, ts(c, TN)], ps[:])

    router_chunk(0)
    xa_sb = {}
    for nm, a_t in (("g", ag), ("u", au)):
        xa = sb.tile([P, T], BF, tag=f"xa_{nm}", name=f"xa_{nm}")
        for c in range(TC):
            ps = psg.tile([P, TN], F32, tag="ps_g", name="ps_xa")
            for k in range(KT):
                nc.tensor.matmul(ps[:], a_t[:, k, :], xTbfh[c][:, k, :],
                                 start=(k == 0), stop=(k == KT - 1))
            nc.scalar.copy(xa[:, ts(c, TN)], ps[:])
        xa_sb[nm] = xa
    router_chunk(1)

    # transpose to L2 packed layout [128, (tile, 16)]
    rn_L2 = sb.tile([P, TT8, 16], F32, tag="rn_L2")
    for tl in range(TT8):
        pst = psm.tile([P, 16], F32, tag="ps_small")
        nc.tensor.transpose(pst[:], rn_sb[:, ts(tl, P)], i128[:16, :16])
        nc.scalar.copy(rn_L2[:, tl, :], pst[:])

    # router_logits = logits + noise * softplus(noise_logits)
    # softplus(x) = ln(exp(x) + 1); |x| is small here so this is accurate and
    # Exp/Ln share one hardware activation table (Softplus has none).
    esp_t = sb.tile([P, TT8, E], F32, tag="esp_t")
    nc.scalar.activation(esp_t[:], rn_L2[:, :, 8:16], AF.Exp)
    sp_t = sb.tile([P, TT8, E], F32, tag="sp_t")
    nc.scalar.activation(sp_t[:], esp_t[:], AF.Ln, bias=1.0)
    nm_t = sb.tile([P, TT8, E], F32, tag="nm_t")
    nc.vector.tensor_tensor(nm_t[:], sp_t[:], noiseL2[:], ALU.mult)
    rl_L2 = sb.tile([P, TT8, E], F32, tag="rl_L2")
    nc.vector.tensor_tensor(rl_L2[:], nm_t[:], rn_L2[:, :, 0:8], ALU.add)
    dma(d["rl"].ap().rearrange("(t p) e -> p t e", p=P), rl_L2[:])

    # softmax top-2 (unnormalized exp works: softmax denominator cancels in
    # both the argmax and the renormalized top-2 weights)
    ex = sb.tile([P, TT8, E], F32, tag="ex")
    nc.scalar.activation(ex[:], rl_L2[:], AF.Exp)
    m1 = sb.tile([P, TT8], F32, tag="m1")
    nc.vector.tensor_reduce(m1[:], ex[:], axis=mybir.AxisListType.X, op=ALU.max)
    # MALL free layout: (tile, mask, e) with mask in {M0, M1, MW0, MW1}
    mall = sb.tile([P, TT8, 4, E], F32, tag="mall")
    m1b = m1[:, :, None].broadcast_to([P, TT8, E])
    nc.vector.tensor_tensor(mall[:, :, 0, :], ex[:], m1b, ALU.is_equal)
    pm = sb.tile([P, TT8, E], F32, tag="pm")
    nc.vector.scalar_tensor_tensor(pm[:], mall[:, :, 0, :], -1e30, ex[:],
                                   op0=ALU.mult, op1=ALU.add)
    m2 = sb.tile([P, TT8], F32, tag="m2")
    nc.vector.tensor_reduce(m2[:], pm[:], axis=mybir.AxisListType.X, op=ALU.max)
    m2b = m2[:, :, None].broadcast_to([P, TT8, E])
    nc.vector.tensor_tensor(mall[:, :, 1, :], pm[:], m2b, ALU.is_equal)
    s12 = sb.tile([P, TT8], F32, tag="s12")
    nc.vector.tensor_tensor(s12[:], m1[:], m2[:], ALU.add)
    r12 = sb.tile([P, TT8], F32, tag="r12")
    nc.vector.reciprocal(r12[:], s12[:])
    w1_ = sb.tile([P, TT8, 2], F32, tag="w1_")  # w per slot
    nc.vector.tensor_tensor(w1_[:, :, 0], m1[:], r12[:], ALU.mult)
    nc.vector.tensor_tensor(w1_[:, :, 1], m2[:], r12[:], ALU.mult)
    for k in range(2):
        wb = w1_[:, :, k][:, :, None].broadcast_to([P, TT8, E])
        nc.vector.tensor_tensor(mall[:, :, 2 + k, :], mall[:, :, k, :], wb, ALU.mult)

    # Runway: gate/up mains for two (f, c=0) pairs are emitted BEFORE the
    # mask transposes so the PE's in-order queue has mask-independent work
    # covering the DVE top-k latency (keeps HAM warm: no >3.4us PE gap).
    def emit_main(which, f, c):
        pool, wt = (psg, wg) if which == "g" else (psu, wu)
        ps = pool.tile([P, TN], F32, tag=f"ps_{which}", name=f"ps_{which}")
        for k in range(KT):
            nc.tensor.matmul(ps[:], wt[:, k, ts(f, P)], xTbfh[c][:, k, :],
                             start=(k == 0), stop=(k == KT - 1))
        mn = stage.tile([P, TN], BF, tag=f"main_{which}", name=f"main_{which}")
        nc.scalar.copy(mn[:], ps[:])  # main only (before the lora matmul)
        return ps, mn

    RUNWAY = [(0, 0), (1, 0)]
    pre = {}
    for f, c in RUNWAY:
        pre["g", f, c] = emit_main("g", f, c)
        pre["u", f, c] = emit_main("u", f, c)

    # transpose masks to L1 and expand over the (e, r) / broadcast axes.
    # M_exp_k[e*R+r, t] = M_k[t, e]; W_exp_k[f, t] = w_k[t].
    mexp = [sb.tile([P, T], BF, tag=f"mexp{k}", name=f"mexp{k}") for k in range(2)]
    wexp = [sb.tile([P, T], BF, tag=f"wexp{k}", name=f"wexp{k}") for k in range(2)]
    dest = [mexp[0], mexp[1], wexp[0], wexp[1]]
    for tl in range(TT8):
        pst = psm.tile([32, P], F32, tag="ps_small")
        nc.tensor.transpose(pst[:], mall[:, tl, :, :], i128[:, :])
        mt4 = stage.tile([32, P], BF, tag="mt4")
        nc.scalar.copy(mt4[:], pst[:])
        for m in range(4):
            pse = psm.tile([P, P], F32, tag="ps_small")
            nc.tensor.matmul(pse[:], emat4[:, ts(m, P)], mt4[:], start=True, stop=True)
            nc.scalar.copy(dest[m][:, ts(tl, P)], pse[:])

    xam = {}
    for nm in ("g", "u"):
        for k in range(2):
            xm = sb.tile([P, T], BF, tag=f"xam_{nm}{k}", name=f"xam_{nm}{k}")
            nc.vector.tensor_tensor(xm[:], xa_sb[nm][:], mexp[k][:], ALU.mult)
            xam[nm, k] = xm

    # ---- D/E/F/G chunk-major: the down matmuls for chunk 0 overlap the
    # gate/up/act chain for chunk 1 ---------------------------------------
    wact = [[sb.tile([P, T], BF, tag=f"wact{k}_{f}", name=f"wact{k}_{f}")
             for f in range(FT)] for k in range(2)]
    zm = [sb.tile([P, T], BF, tag=f"zm{k}", name=f"zm{k}") for k in range(2)]
    actw = [sb.tile([P, T], BF, tag=f"actw_{f}", name=f"actw_{f}") for f in range(FT)]

    def emit_lora0(which, f, c, ps):
        # slot-0 lora accumulates in PSUM on top of main (group reopened)
        b_t = bg if which == "g" else bu
        nc.tensor.matmul(ps[:], b_t[:, ts(f, P)], xam[which, 0][:, ts(c, TN)],
                         start=False, stop=True, skip_group_check=True)

    def emit_lora1(which, f, c, mn):
        b_t = bg if which == "g" else bu
        psl = psm.tile([P, TN], F32, tag="ps_small", name="ps_l")
        nc.tensor.matmul(psl[:], b_t[:, ts(f, P)], xam[which, 1][:, ts(c, TN)],
                         start=True, stop=True)
        t1 = stage.tile([P, TN], BF, tag=f"{which}b1", name=f"{which}b1")
        nc.vector.tensor_tensor(t1[:], psl[:], mn[:], ALU.add)
        return t1

    for c in range(TC):
        for f in range(FT):
            psG, mainG = pre.pop(("g", f, c)) if ("g", f, c) in pre else emit_main("g", f, c)
            emit_lora0("g", f, c, psG)
            psU, mainU = pre.pop(("u", f, c)) if ("u", f, c) in pre else emit_main("u", f, c)
            emit_lora0("u", f, c, psU)
            gb1 = emit_lora1("g", f, c, mainG)
            ub1 = emit_lora1("u", f, c, mainU)
            # slot 0: silu straight from PSUM (one ACT op; no sigmoid-mul)
            sil0 = stage.tile([P, TN], BF, tag="sig")
            nc.scalar.activation(sil0[:], psG[:], _silu_fn())
            if not USE_SILU:
                t = stage.tile([P, TN], BF, tag="gs", name="gs0t")
                nc.vector.tensor_tensor(t[:], psG[:], sil0[:], ALU.mult)
                sil0 = t
            uw0 = stage.tile([P, TN], BF, tag="uw")
            nc.vector.tensor_tensor(uw0[:], psU[:], wexp[0][:, ts(c, TN)], ALU.mult)
            nc.vector.tensor_tensor(wact[0][f][:, ts(c, TN)], sil0[:], uw0[:], ALU.mult)
            # slot 1
            sil1 = stage.tile([P, TN], BF, tag="sig")
            nc.scalar.activation(sil1[:], gb1[:], _silu_fn())
            if not USE_SILU:
                t = stage.tile([P, TN], BF, tag="gs", name="gs1t")
                nc.vector.tensor_tensor(t[:], gb1[:], sil1[:], ALU.mult)
                sil1 = t
            uw1 = stage.tile([P, TN], BF, tag="uw")
            nc.vector.tensor_tensor(uw1[:], ub1[:], wexp[1][:, ts(c, TN)], ALU.mult)
            nc.vector.tensor_tensor(wact[1][f][:, ts(c, TN)], sil1[:], uw1[:], ALU.mult)
        # z_k = wact_k @ Ad, masked (this chunk)
        for k in range(2):
            ps = psm.tile([P, TN], F32, tag="ps_small", name="ps_z")
            for f in range(FT):
                nc.tensor.matmul(ps[:], ad[:, f, :], wact[k][f][:, ts(c, TN)],
                                 start=(f == 0), stop=(f == FT - 1))
            nc.vector.tensor_tensor(zm[k][:, ts(c, TN)], ps[:], mexp[k][:, ts(c, TN)], ALU.mult)
        # fold slots
        for f in range(FT):
            nc.vector.tensor_tensor(actw[f][:, ts(c, TN)], wact[0][f][:, ts(c, TN)],
                                    wact[1][f][:, ts(c, TN)], ALU.add)
        # down projection + lora (this chunk)
        for h in range(HT):
            ps = pso.tile([P, TN], F32, tag="ps_o", name="ps_o")
            for f in range(FT):
                nc.tensor.matmul(ps[:], wd[:, f, ts(h, P)], actw[f][:, ts(c, TN)],
                                 start=(f == 0), stop=False)
            nc.tensor.matmul(ps[:], bd[:, ts(h, P)], zm[0][:, ts(c, TN)],
                             start=False, stop=False)
            nc.tensor.matmul(ps[:], bd[:, ts(h, P)], zm[1][:, ts(c, TN)],
                             start=False, stop=True)
            ot = stage.tile([P, TN], F32, tag="ot")
            nc.scalar.copy(ot[:], ps[:])  # ACT is idle during the down phase
            dma(d["outT"].ap()[ts(h, P), ts(c, TN)], ot[:])


def _build():
    nc = bacc.Bacc("TRN2", target_bir_lowering=False, debug=False)
    d = {}
    d["xT32"] = nc.dram_tensor("xT32", [H, T], F32, kind="ExternalInput")
    d["xTbf"] = nc.dram_tensor("xTbf", [H, T], BF, kind="ExternalInput")
    d["wrn"] = nc.dram_tensor("wrn", [H, 16], F32, kind="ExternalInput")
    d["wg"] = nc.dram_tensor("wg", [H, FC], BF, kind="ExternalInput")
    d["wu"] = nc.dram_tensor("wu", [H, FC], BF, kind="ExternalInput")
    d["wd"] = nc.dram_tensor("wd", [FC, H], BF, kind="ExternalInput")
    d["ag"] = nc.dram_tensor("ag", [H, ER], BF, kind="ExternalInput")
    d["au"] = nc.dram_tensor("au", [H, ER], BF, kind="ExternalInput")
    d["bg"] = nc.dram_tensor("bg", [ER, FC], BF, kind="ExternalInput")
    d["bu"] = nc.dram_tensor("bu", [ER, FC], BF, kind="ExternalInput")
    d["ad"] = nc.dram_tensor("ad", [FC, ER], BF, kind="ExternalInput")
    d["bd"] = nc.dram_tensor("bd", [ER, H], BF, kind="ExternalInput")
    d["noiseL2"] = nc.dram_tensor("noiseL2", [P, TT8 * E], F32, kind="ExternalInput")
    d["emat4"] = nc.dram_tensor("emat4", [32, 4 * P], BF, kind="ExternalInput")
    d["i128"] = nc.dram_tensor("i128", [P, P], F32, kind="ExternalInput")
    d["outT"] = nc.dram_tensor("outT", [H, T], F32, kind="ExternalOutput")
    d["rl"] = nc.dram_tensor("rl", [T, E], F32, kind="ExternalOutput")

    with tile.TileContext(nc) as tc, ExitStack() as ctx:
        _emit(nc, tc, ctx, d)
    nc.compile()
    return nc


_NC = None


def _get_nc():
    global _NC
    if _NC is None:
        _NC = _build()
    return _NC


def _emat4_np():
    m = np.zeros((32, 4 * P), dtype=np.float32)
    for blk in range(4):
        for e in range(E):
            if blk < 2:
                m[blk * 8 + e, blk * P + e * R:blk * P + (e + 1) * R] = 1.0
            else:
                m[blk * 8 + e, blk * P:(blk + 1) * P] = 1.0
    return m.astype(BF16)


def make_in_maps(hidden_states, noise, w_route, w_noise, w_gate, w_up, w_down,
                 a_gate, b_gate, a_up, b_up, a_down, b_down):
    x = np.asarray(hidden_states, dtype=np.float32).reshape(T, H)
    xT = np.ascontiguousarray(x.T)
    shared = {
        "xT32": xT,
        "xTbf": xT.astype(BF16),
        "wrn": np.ascontiguousarray(
            np.concatenate([w_route, w_noise], axis=1).astype(np.float32)),
        "ag": np.ascontiguousarray(
            a_gate.transpose(1, 0, 2).reshape(H, ER)).astype(BF16),
        "au": np.ascontiguousarray(
            a_up.transpose(1, 0, 2).reshape(H, ER)).astype(BF16),
        "bd": np.ascontiguousarray(
            (SCALING * b_down).reshape(ER, H)).astype(BF16),
        "noiseL2": np.ascontiguousarray(
            np.asarray(noise, np.float32).reshape(TT8, P, E)
            .transpose(1, 0, 2).reshape(P, TT8 * E)),
        "emat4": _emat4_np(),
        "i128": np.eye(P, dtype=np.float32),
    }
    bg2 = (SCALING * b_gate).reshape(ER, F)
    bu2 = (SCALING * b_up).reshape(ER, F)
    adf = a_down.transpose(1, 0, 2).reshape(F, ER)
    in_maps = []
    for c in range(NCORES):
        fsl = slice(c * FC, (c + 1) * FC)
        m = dict(shared)
        m["wg"] = np.ascontiguousarray(w_gate[:, fsl]).astype(BF16)
        m["wu"] = np.ascontiguousarray(w_up[:, fsl]).astype(BF16)
        m["wd"] = np.ascontiguousarray(w_down[fsl, :]).astype(BF16)
        m["bg"] = np.ascontiguousarray(bg2[:, fsl]).astype(BF16)
        m["bu"] = np.ascontiguousarray(bu2[:, fsl]).astype(BF16)
        m["ad"] = np.ascontiguousarray(adf[fsl, :]).astype(BF16)
        in_maps.append(m)
    return in_maps


def kernel(hidden_states, noise, w_route, w_noise, w_gate, w_up, w_down,
           a_gate, b_gate, a_up, b_up, a_down, b_down, _trace=False):
    in_maps = make_in_maps(hidden_states, noise, w_route, w_noise, w_gate,
                           w_up, w_down, a_gate, b_gate, a_up, b_up,
                           a_down, b_down)
    nc = _get_nc()
    res = run_bass_kernel_spmd(nc, in_maps, list(range(NCORES)), trace=_trace)
    outs = res.results
    outT = np.zeros((H, T), dtype=np.float32)
    for o in outs:
        outT += o["outT"]
    out = np.ascontiguousarray(outT.T).reshape(B, S, H)
    rl = np.ascontiguousarray(outs[0]["rl"]).astype(np.float32)
    if _trace:
        kernel._last_exec_time_ns = res.exec_time_ns
    return out, rl


# revision 23
# speedup vs baseline: 1.1356x; 1.0081x over previous
"""Trainium2 Bass kernel for nn_LoraMoeBlock (MoE with per-expert LoRA adapters).

Strategy
--------
Math reformulation ("slot-mask" form): with top-k=2, write the combine as a sum
over two slots k in {0,1}; slot k of token t selects expert e_k(t) with weight
w_k(t).  Per-expert LoRA terms become dense matmuls over the packed [E*R=128]
axis by masking the small per-token LoRA activations:

    xa      = x @ A            (A packed [H, E*R]) -- all experts at once
    xam_k   = xa * M_k         (M_k[t, e*R+r] = 1 iff e == e_k(t))
    gate_k  = x @ Wg + xam_gk @ Bg      (PSUM accumulation, Bg pre-scaled by 2)
    up_k    = x @ Wu + xam_uk @ Bu
    wact_k  = w_k * silu(gate_k) * up_k
    actw    = wact_0 + wact_1
    z_k     = wact_k @ Ad ;  zm_k = z_k * M_k
    out     = actw @ Wd + zm_0 @ Bd + zm_1 @ Bd   (PSUM accumulation)

Everything is a dense matmul + a handful of elementwise passes over [T, Fc]
per slot; exact top-2 routing semantics are preserved via the masks, which are
computed on-device from an fp32 router (selection is precision sensitive).

Sharding: the F=4096 axis is split across the 8 cores (Fc=512 each).  Weights
touching F are sliced per core; x, router and LoRA-A weights are replicated.
Each core produces a partial out^T [H, T] (partial over its F slice, since the
down-projection contracts over F) plus the router logits; the host sums the 8
partials as the unshard step (the contraction-sharded equivalent of gather).

On-device layout is "L1": feature-on-partition, token-on-free ([*, T] tiles),
which lets every matmul consume weights in their natural layout with x^T (host
provides the transpose) as the shared moving operand.  The small router /
top-k stage runs in an L2 packed layout [128, (tile, e)] where softmax and
top-2 are free-dim reductions; masks hop back to L1 via one PE transpose and a
constant "expand" matmul per token-tile.

Matmul inputs are bf16 (full PE rate; fp32 matmul is 4x slower on trn2); the
router matmul is fp32 to make expert selection bit-reliable.  All PSUM
accumulation is fp32.
"""

import sys

sys.path.insert(0, "/opt/trn_rl_repo")

from contextlib import ExitStack

import ml_dtypes
import numpy as np

import concourse.bass as bass
import concourse.tile as tile
from concourse import bacc, mybir
from concourse.bass import ts
from concourse.bass_utils import run_bass_kernel_spmd

BF16 = ml_dtypes.bfloat16
F32 = mybir.dt.float32
BF = mybir.dt.bfloat16
AF = mybir.ActivationFunctionType
ALU = mybir.AluOpType

USE_SILU = True  # HW has a Silu table; CoreSim doesn't implement it, so
                 # sim_check flips this to False (x*sigmoid(x), same math)


def _silu_fn():
    return AF.Silu if USE_SILU else AF.Sigmoid


B, S, H, F, E, R = 1, 1024, 1024, 4096, 8, 16
T = B * S
ER = E * R  # 128
NCORES = 8
FC = F // NCORES  # 512 per-core F slice
SCALING = 2.0

P = 128
KT = H // P  # 8 contraction tiles over H
FT = FC // P  # 4 partition tiles over Fc
HT = H // P  # 8 output tiles over H
TT8 = T // P  # 8 token tiles
TN = 512  # moving-operand chunk
TC = T // TN  # 2 chunks


def _emit(nc, tc, ctx, d):
    """Emit the per-core program. d: dict of dram tensor handles."""
    PS = bass.MemorySpace.PSUM

    consts = ctx.enter_context(tc.tile_pool(name="consts", bufs=1))
    sb = ctx.enter_context(tc.tile_pool(name="sb", bufs=1))
    stage = ctx.enter_context(tc.tile_pool(name="stage", bufs=4))
    psg = ctx.enter_context(tc.tile_pool(name="psg", bufs=2, space=PS))
    psu = ctx.enter_context(tc.tile_pool(name="psu", bufs=2, space=PS))
    pso = ctx.enter_context(tc.tile_pool(name="pso", bufs=2, space=PS))
    psm = ctx.enter_context(tc.tile_pool(name="psm", bufs=2, space=PS))

    dma = nc.sync.dma_start

    # ---- constant loads -------------------------------------------------
    def cload(name, shape, dt, ap):
        t_ = consts.tile(shape, dt, tag=name, name=name)
        dma(t_[:], ap)
        return t_

    # DMA order = consumption order. The router is the head of the mask
    # critical path, so its inputs stream first (xT32 split into T-halves so
    # the first router chunk starts after ~2MB); xa/mains fill PE behind it.
    # x^T halves are separate tiles so chunk-0 compute only waits on the
    # chunk-0 DMA (one tile written by two DMAs would serialize on both)
    wrn = cload("wrn", [P, KT, 16], F32, d["wrn"].ap().rearrange("(k p) e -> p k e", p=P))
    xT32_src = d["xT32"].ap().rearrange("(k p) t -> p k t", p=P)
    xTbf_src = d["xTbf"].ap().rearrange("(k p) t -> p k t", p=P)
    xT32h, xTbfh = [None, None], [None, None]
    xT32h[0] = cload("xT32_0", [P, KT, TN], F32, xT32_src[:, :, ts(0, TN)])
    xTbfh[0] = cload("xTbf_0", [P, KT, TN], BF, xTbf_src[:, :, ts(0, TN)])
    xT32h[1] = cload("xT32_1", [P, KT, TN], F32, xT32_src[:, :, ts(1, TN)])
    xTbfh[1] = cload("xTbf_1", [P, KT, TN], BF, xTbf_src[:, :, ts(1, TN)])
    ag = cload("ag", [P, KT, ER], BF, d["ag"].ap().rearrange("(k p) r -> p k r", p=P))
    au = cload("au", [P, KT, ER], BF, d["au"].ap().rearrange("(k p) r -> p k r", p=P))
    noiseL2 = cload("noiseL2", [P, TT8, E], F32, d["noiseL2"].ap().rearrange("p (t e) -> p t e", e=E))
    emat4 = cload("emat4", [32, 4 * P], BF, d["emat4"].ap())
    i128 = cload("i128", [P, P], F32, d["i128"].ap())
    wg = cload("wg", [P, KT, FC], BF, d["wg"].ap().rearrange("(k p) f -> p k f", p=P))
    wu = cload("wu", [P, KT, FC], BF, d["wu"].ap().rearrange("(k p) f -> p k f", p=P))
    bg = cload("bg", [P, FC], BF, d["bg"].ap())
    bu = cload("bu", [P, FC], BF, d["bu"].ap())
    ad = cload("ad", [P, FT, ER], BF, d["ad"].ap().rearrange("(k p) r -> p k r", p=P))
    wd = cload("wd", [P, FT, H], BF, d["wd"].ap().rearrange("(k p) h -> p k h", p=P))
    bd = cload("bd", [P, H], BF, d["bd"].ap())

    # ---- B: router matmuls (fp32) + C1: xa (PE fill work) ----------------
    rn_sb = sb.tile([16, T], F32, tag="rn_sb")

    def router_chunk(c):
        ps = psm.tile([16, TN], F32, tag="ps_small", name="ps_rn")
        for k in range(KT):
            nc.tensor.matmul(ps[:], wrn[:, k, :], xT32h[c][:, k, :],
                             start=(k == 0), stop=(k == KT - 1))
        nc.scalar.copy(rn_sb[:, ts(c, TN)], ps[:])

    router_chunk(0)
    xa_sb = {}
    for nm, a_t in (("g", ag), ("u", au)):
        xa = sb.tile([P, T], BF, tag=f"xa_{nm}", name=f"xa_{nm}")
        for c in range(TC):
            ps = psg.tile([P, TN], F32, tag="ps_g", name="ps_xa")
            for k in range(KT):
                nc.tensor.matmul(ps[:], a_t[:, k, :], xTbfh[c][:, k, :],
                                 start=(k == 0), stop=(k == KT - 1))
            nc.scalar.copy(xa[:, ts(c, TN)], ps[:])
        xa_sb[nm] = xa
    router_chunk(1)

    # transpose to L2 packed layout [128, (tile, 16)]
    rn_L2 = sb.tile([P, TT8, 16], F32, tag="rn_L2")
    for tl in range(TT8):
        pst = psm.tile([P, 16], F32, tag="ps_small")
        nc.tensor.transpose(pst[:], rn_sb[:, ts(tl, P)], i128[:16, :16])
        nc.scalar.copy(rn_L2[:, tl, :], pst[:])

    # router_logits = logits + noise * softplus(noise_logits)
    # softplus(x) = ln(exp(x) + 1); |x| is small here so this is accurate and
    # Exp/Ln share one hardware activation table (Softplus has none).
    esp_t = sb.tile([P, TT8, E], F32, tag="esp_t")
    nc.scalar.activation(esp_t[:], rn_L2[:, :, 8:16], AF.Exp)
    sp_t = sb.tile([P, TT8, E], F32, tag="sp_t")
    nc.scalar.activation(sp_t[:], esp_t[:], AF.Ln, bias=1.0)
    nm_t = sb.tile([P, TT8, E], F32, tag="nm_t")
    nc.vector.tensor_tensor(nm_t[:], sp_t[:], noiseL2[:], ALU.mult)
    rl_L2 = sb.tile([P, TT8, E], F32, tag="rl_L2")
    nc.vector.tensor_tensor(rl_L2[:], nm_t[:], rn_L2[:, :, 0:8], ALU.add)
    dma(d["rl"].ap().rearrange("(t p) e -> p t e", p=P), rl_L2[:])

    # softmax top-2 (unnormalized exp works: softmax denominator cancels in
    # both the argmax and the renormalized top-2 weights)
    ex = sb.tile([P, TT8, E], F32, tag="ex")
    nc.scalar.activation(ex[:], rl_L2[:], AF.Exp)
    m1 = sb.tile([P, TT8], F32, tag="m1")
    nc.vector.tensor_reduce(m1[:], ex[:], axis=mybir.AxisListType.X, op=ALU.max)
    # MALL free layout: (tile, mask, e) with mask in {M0, M1, MW0, MW1}
    mall = sb.tile([P, TT8, 4, E], F32, tag="mall")
    m1b = m1[:, :, None].broadcast_to([P, TT8, E])
    nc.vector.tensor_tensor(mall[:, :, 0, :], ex[:], m1b, ALU.is_equal)
    pm = sb.tile([P, TT8, E], F32, tag="pm")
    nc.vector.scalar_tensor_tensor(pm[:], mall[:, :, 0, :], -1e30, ex[:],
                                   op0=ALU.mult, op1=ALU.add)
    m2 = sb.tile([P, TT8], F32, tag="m2")
    nc.vector.tensor_reduce(m2[:], pm[:], axis=mybir.AxisListType.X, op=ALU.max)
    m2b = m2[:, :, None].broadcast_to([P, TT8, E])
    nc.vector.tensor_tensor(mall[:, :, 1, :], pm[:], m2b, ALU.is_equal)
    s12 = sb.tile([P, TT8], F32, tag="s12")
    nc.vector.tensor_tensor(s12[:], m1[:], m2[:], ALU.add)
    r12 = sb.tile([P, TT8], F32, tag="r12")
    nc.vector.reciprocal(r12[:], s12[:])
    w1_ = sb.tile([P, TT8, 2], F32, tag="w1_")  # w per slot
    nc.vector.tensor_tensor(w1_[:, :, 0], m1[:], r12[:], ALU.mult)
    nc.vector.tensor_tensor(w1_[:, :, 1], m2[:], r12[:], ALU.mult)
    for k in range(2):
        wb = w1_[:, :, k][:, :, None].broadcast_to([P, TT8, E])
        nc.vector.tensor_tensor(mall[:, :, 2 + k, :], mall[:, :, k, :], wb, ALU.mult)

    # Runway: gate/up mains for two (f, c=0) pairs are emitted BEFORE the
    # mask transposes so the PE's in-order queue has mask-independent work
    # covering the DVE top-k latency (keeps HAM warm: no >3.4us PE gap).
    def emit_main(which, f, c):
        pool, wt = (psg, wg) if which == "g" else (psu, wu)
        ps = pool.tile([P, TN], F32, tag=f"ps_{which}", name=f"ps_{which}")
        for k in range(KT):
            nc.tensor.matmul(ps[:], wt[:, k, ts(f, P)], xTbfh[c][:, k, :],
                             start=(k == 0), stop=(k == KT - 1))
        mn = stage.tile([P, TN], BF, tag=f"main_{which}", name=f"main_{which}")
        nc.scalar.copy(mn[:], ps[:])  # main only (before the lora matmul)
        return ps, mn

    RUNWAY = [(0, 0), (1, 0)]
    pre = {}
    for f, c in RUNWAY:
        pre["g", f, c] = emit_main("g", f, c)
        pre["u", f, c] = emit_main("u", f, c)

    # transpose masks to L1 and expand over the (e, r) / broadcast axes.
    # M_exp_k[e*R+r, t] = M_k[t, e]; W_exp_k[f, t] = w_k[t].
    mexp = [sb.tile([P, T], BF, tag=f"mexp{k}", name=f"mexp{k}") for k in range(2)]
    wexp = [sb.tile([P, T], BF, tag=f"wexp{k}", name=f"wexp{k}") for k in range(2)]
    dest = [mexp[0], mexp[1], wexp[0], wexp[1]]
    for tl in range(TT8):
        pst = psm.tile([32, P], F32, tag="ps_small")
        nc.tensor.transpose(pst[:], mall[:, tl, :, :], i128[:, :])
        mt4 = stage.tile([32, P], BF, tag="mt4")
        nc.scalar.copy(mt4[:], pst[:])
        for m in range(4):
            pse = psm.tile([P, P], F32, tag="ps_small")
            nc.tensor.matmul(pse[:], emat4[:, ts(m, P)], mt4[:], start=True, stop=True)
            nc.scalar.copy(dest[m][:, ts(tl, P)], pse[:])

    xam = {}
    for nm in ("g", "u"):
        for k in range(2):
            xm = sb.tile([P, T], BF, tag=f"xam_{nm}{k}", name=f"xam_{nm}{k}")
            nc.vector.tensor_tensor(xm[:], xa_sb[nm][:], mexp[k][:], ALU.mult)
            xam[nm, k] = xm

    # ---- D/E/F/G chunk-major: the down matmuls for chunk 0 overlap the
    # gate/up/act chain for chunk 1 ---------------------------------------
    wact = [[sb.tile([P, T], BF, tag=f"wact{k}_{f}", name=f"wact{k}_{f}")
             for f in range(FT)] for k in range(2)]
    zm = [sb.tile([P, T], BF, tag=f"zm{k}", name=f"zm{k}") for k in range(2)]
    actw = [sb.tile([P, T], BF, tag=f"actw_{f}", name=f"actw_{f}") for f in range(FT)]

    def emit_lora0(which, f, c, ps):
        # slot-0 lora accumulates in PSUM on top of main (group reopened)
        b_t = bg if which == "g" else bu
        nc.tensor.matmul(ps[:], b_t[:, ts(f, P)], xam[which, 0][:, ts(c, TN)],
                         start=False, stop=True, skip_group_check=True)

    def emit_lora1(which, f, c, mn):
        b_t = bg if which == "g" else bu
        psl = psm.tile([P, TN], F32, tag="ps_small", name="ps_l")
        nc.tensor.matmul(psl[:], b_t[:, ts(f, P)], xam[which, 1][:, ts(c, TN)],
                         start=True, stop=True)
        t1 = stage.tile([P, TN], BF, tag=f"{which}b1", name=f"{which}b1")
        nc.vector.tensor_tensor(t1[:], psl[:], mn[:], ALU.add)
        return t1

    for c in range(TC):
        for f in range(FT):
            psG, mainG = pre.pop(("g", f, c)) if ("g", f, c) in pre else emit_main("g", f, c)
            emit_lora0("g", f, c, psG)
            psU, mainU = pre.pop(("u", f, c)) if ("u", f, c) in pre else emit_main("u", f, c)
            emit_lora0("u", f, c, psU)
            gb1 = emit_lora1("g", f, c, mainG)
            ub1 = emit_lora1("u", f, c, mainU)
            # slot 0: silu straight from PSUM (one ACT op; no sigmoid-mul)
            sil0 = stage.tile([P, TN], BF, tag="sig")
            nc.scalar.activation(sil0[:], psG[:], _silu_fn())
            if not USE_SILU:
                t = stage.tile([P, TN], BF, tag="gs", name="gs0t")
                nc.vector.tensor_tensor(t[:], psG[:], sil0[:], ALU.mult)
                sil0 = t
            uw0 = stage.tile([P, TN], BF, tag="uw")
            nc.vector.tensor_tensor(uw0[:], psU[:], wexp[0][:, ts(c, TN)], ALU.mult)
            nc.vector.tensor_tensor(wact[0][f][:, ts(c, TN)], sil0[:], uw0[:], ALU.mult)
            # slot 1
            sil1 = stage.tile([P, TN], BF, tag="sig")
            nc.scalar.activation(sil1[:], gb1[:], _silu_fn())
            if not USE_SILU:
                t = stage.tile([P, TN], BF, tag="gs", name="gs1t")
                nc.vector.tensor_tensor(t[:], gb1[:], sil1[:], ALU.mult)
                sil1 = t
            uw1 = stage.tile([P, TN], BF, tag="uw")
            nc.vector.tensor_tensor(uw1[:], ub1[:], wexp[1][:, ts(c, TN)], ALU.mult)
            nc.vector.tensor_tensor(wact[1][f][:, ts(c, TN)], sil1[:], uw1[:], ALU.mult)
        # z_k = wact_k @ Ad, masked (this chunk); zm0+zm1 fold into one Bd
        # matmul since Bd is shared between the slots
        for k in range(2):
            ps = psm.tile([P, TN], F32, tag="ps_small", name="ps_z")
            for f in range(FT):
                nc.tensor.matmul(ps[:], ad[:, f, :], wact[k][f][:, ts(c, TN)],
                                 start=(f == 0), stop=(f == FT - 1))
            if k == 0:
                nc.vector.tensor_tensor(zm[0][:, ts(c, TN)], ps[:],
                                        mexp[0][:, ts(c, TN)], ALU.mult)
            else:
                zt = stage.tile([P, TN], BF, tag="zt")
                nc.vector.tensor_tensor(zt[:], ps[:], mexp[1][:, ts(c, TN)], ALU.mult)
                nc.vector.tensor_tensor(zm[0][:, ts(c, TN)], zm[0][:, ts(c, TN)],
                                        zt[:], ALU.add)
        # fold slots
        for f in range(FT):
            nc.vector.tensor_tensor(actw[f][:, ts(c, TN)], wact[0][f][:, ts(c, TN)],
                                    wact[1][f][:, ts(c, TN)], ALU.add)
        # runway: next chunk's first main groups keep PE fed while the down
        # inputs (actw/zm, DVE-produced) finalize
        if c + 1 < TC:
            for f2 in (0, 1):
                pre["g", f2, c + 1] = emit_main("g", f2, c + 1)
                pre["u", f2, c + 1] = emit_main("u", f2, c + 1)
        # down projection + lora (this chunk)
        for h in range(HT):
            ps = pso.tile([P, TN], F32, tag="ps_o", name="ps_o")
            for f in range(FT):
                nc.tensor.matmul(ps[:], wd[:, f, ts(h, P)], actw[f][:, ts(c, TN)],
                                 start=(f == 0), stop=False)
            nc.tensor.matmul(ps[:], bd[:, ts(h, P)], zm[0][:, ts(c, TN)],
                             start=False, stop=True)
            ot = stage.tile([P, TN], F32, tag="ot")
            nc.scalar.copy(ot[:], ps[:])  # ACT is idle during the down phase
            dma(d["outT"].ap()[ts(h, P), ts(c, TN)], ot[:])


def _build():
    nc = bacc.Bacc("TRN2", target_bir_lowering=False, debug=False)
    d = {}
    d["xT32"] = nc.dram_tensor("xT32", [H, T], F32, kind="ExternalInput")
    d["xTbf"] = nc.dram_tensor("xTbf", [H, T], BF, kind="ExternalInput")
    d["wrn"] = nc.dram_tensor("wrn", [H, 16], F32, kind="ExternalInput")
    d["wg"] = nc.dram_tensor("wg", [H, FC], BF, kind="ExternalInput")
    d["wu"] = nc.dram_tensor("wu", [H, FC], BF, kind="ExternalInput")
    d["wd"] = nc.dram_tensor("wd", [FC, H], BF, kind="ExternalInput")
    d["ag"] = nc.dram_tensor("ag", [H, ER], BF, kind="ExternalInput")
    d["au"] = nc.dram_tensor("au", [H, ER], BF, kind="ExternalInput")
    d["bg"] = nc.dram_tensor("bg", [ER, FC], BF, kind="ExternalInput")
    d["bu"] = nc.dram_tensor("bu", [ER, FC], BF, kind="ExternalInput")
    d["ad"] = nc.dram_tensor("ad", [FC, ER], BF, kind="ExternalInput")
    d["bd"] = nc.dram_tensor("bd", [ER, H], BF, kind="ExternalInput")
    d["noiseL2"] = nc.dram_tensor("noiseL2", [P, TT8 * E], F32, kind="ExternalInput")
    d["emat4"] = nc.dram_tensor("emat4", [32, 4 * P], BF, kind="ExternalInput")
    d["i128"] = nc.dram_tensor("i128", [P, P], F32, kind="ExternalInput")
    d["outT"] = nc.dram_tensor("outT", [H, T], F32, kind="ExternalOutput")
    d["rl"] = nc.dram_tensor("rl", [T, E], F32, kind="ExternalOutput")

    with tile.TileContext(nc) as tc, ExitStack() as ctx:
        _emit(nc, tc, ctx, d)
    nc.compile()
    return nc


_NC = None


def _get_nc():
    global _NC
    if _NC is None:
        _NC = _build()
    return _NC


def _emat4_np():
    m = np.zeros((32, 4 * P), dtype=np.float32)
    for blk in range(4):
        for e in range(E):
            if blk < 2:
                m[blk * 8 + e, blk * P + e * R:blk * P + (e + 1) * R] = 1.0
            else:
                m[blk * 8 + e, blk * P:(blk + 1) * P] = 1.0
    return m.astype(BF16)


def make_in_maps(hidden_states, noise, w_route, w_noise, w_gate, w_up, w_down,
                 a_gate, b_gate, a_up, b_up, a_down, b_down):
    x = np.asarray(hidden_states, dtype=np.float32).reshape(T, H)
    xT = np.ascontiguousarray(x.T)
    shared = {
        "xT32": xT,
        "xTbf": xT.astype(BF16),
        "wrn": np.ascontiguousarray(
            np.concatenate([w_route, w_noise], axis=1).astype(np.float32)),
        "ag": np.ascontiguousarray(
            a_gate.transpose(1, 0, 2).reshape(H, ER)).astype(BF16),
        "au": np.ascontiguousarray(
            a_up.transpose(1, 0, 2).reshape(H, ER)).astype(BF16),
        "bd": np.ascontiguousarray(
            (SCALING * b_down).reshape(ER, H)).astype(BF16),
        "noiseL2": np.ascontiguousarray(
            np.asarray(noise, np.float32).reshape(TT8, P, E)
            .transpose(1, 0, 2).reshape(P, TT8 * E)),
        "emat4": _emat4_np(),
        "i128": np.eye(P, dtype=np.float32),
    }
    bg2 = (SCALING * b_gate).reshape(ER, F)
    bu2 = (SCALING * b_up).reshape(ER, F)
    adf = a_down.transpose(1, 0, 2).reshape(F, ER)
    in_maps = []
    for c in range(NCORES):
        fsl = slice(c * FC, (c + 1) * FC)
        m = dict(shared)
        m["wg"] = np.ascontiguousarray(w_gate[:, fsl]).astype(BF16)
        m["wu"] = np.ascontiguousarray(w_up[:, fsl]).astype(BF16)
        m["wd"] = np.ascontiguousarray(w_down[fsl, :]).astype(BF16)
        m["bg"] = np.ascontiguousarray(bg2[:, fsl]).astype(BF16)
        m["bu"] = np.ascontiguousarray(bu2[:, fsl]).astype(BF16)
        m["ad"] = np.ascontiguousarray(adf[fsl, :]).astype(BF16)
        in_maps.append(m)
    return in_maps


def kernel(hidden_states, noise, w_route, w_noise, w_gate, w_up, w_down,
           a_gate, b_gate, a_up, b_up, a_down, b_down, _trace=False):
    in_maps = make_in_maps(hidden_states, noise, w_route, w_noise, w_gate,
                           w_up, w_down, a_gate, b_gate, a_up, b_up,
                           a_down, b_down)
    nc = _get_nc()
    res = run_bass_kernel_spmd(nc, in_maps, list(range(NCORES)), trace=_trace)
    outs = res.results
    outT = np.zeros((H, T), dtype=np.float32)
    for o in outs:
        outT += o["outT"]
    out = np.ascontiguousarray(outT.T).reshape(B, S, H)
    rl = np.ascontiguousarray(outs[0]["rl"]).astype(np.float32)
    if _trace:
        kernel._last_exec_time_ns = res.exec_time_ns
    return out, rl
